# revision 16
# baseline (speedup 1.0000x reference)
"""Trainium2 Bass kernel for nn_BiLinear (synthetic EMLP BiLinear).

Math: out[b,o] = 0.05 * sum_i x[b,i] * Wflat[b, perm[o*512+i]]
where Wflat[b,k] is a small GEMM of param rows against gathered x columns:
  k < M0:  Wflat[b,k] = sum_{n<128} p0[k,n] * x[b, bids0[n]]
  k >= M0: (m,s) = divmod(k-M0,12); Wflat[b,k] = sum_{n<32} p1[m,n] * x[b, bids1[n*12+s]]

Since perm is a permutation of the full 512x512 (o,i) grid, we host-reorder the
param rows into "grid order": pgrid[:, o*512+i] holds the param row of cell
(o,i), scattered into a unified 512-tall contraction space
  q in [0,128)          -> gathered col bids0[q]
  q = 128 + s*32 + n    -> gathered col bids1[n*12+s]
On device (per o-row, per batch-half):
  V[b, i] = sum_q xg[q,b] * pgrid[q, (o,i)]   (accumulating matmuls)
  out[b,o] = scale * sum_i V[b,(o,i)] * x[b,i] (fused mul+reduce, balanced
                                                across DVE and Pool engines)
xg (the gathered-x operand) is host-computed and uploaded directly.

Modes:
  f16  : xg/pgrid in fp16, 4 accumulating matmuls per (o,half).
  dr8  : xg/pgrid in fp8 e4m3 (params pre-scaled x4096), chunk pairs packed
         into DoubleRow matmuls -> 2 matmuls per (o,half) at 0.5 cyc/col,
         and half the pgrid HBM traffic vs f16.
  dr8c : like dr8 but xg split into fp8 hi+lo for error compensation
         (4 DoubleRow matmuls per (o,half)).

Sharding: output rows o are split across the 8 cores (64 each). x and xg are
replicated; pgrid is partitioned. No collectives; the host concatenates the
per-core (256, 64) outputs.
"""

import os
import sys

import numpy as np

if "/opt/trn_rl_repo" not in sys.path:
    sys.path.insert(0, "/opt/trn_rl_repo")

# Problem constants (hardcoded per contract).
S0, S1 = 1, 12
N0, N1 = 128, 32
M0, M1 = 22144, 20000
DIN, DOUT = 512, 512
WSIZE = DOUT * DIN
B = 256
NCORES = 8
OSH = DOUT // NCORES  # output rows per core
KCH = 4  # contraction chunks of 128
CELLS = OSH * DIN  # grid cells per core

PG_SCALE = {"f16": 1.0, "dr8": 4096.0, "dr8c": 4096.0, "mix": 4096.0}
_DT_MODE = os.environ.get("KERNEL_DTYPE", "ef2")
# ef2 tuning knobs
_EF2_WARM = int(os.environ.get("EF2_WARM", "160"))  # warmup matmuls (N=64)
_EF2_DVE_OF = int(os.environ.get("EF2_DVE_OF", "5"))  # DVE units per MOD
_EF2_DVE_MOD = int(os.environ.get("EF2_DVE_MOD", "8"))
_EF2_CDEPTH = int(os.environ.get("EF2_CDEPTH", "7"))  # stage-C pipeline depth
# In mix mode, rows with (o % 16) < _HILO16 use hi/lo-compensated fp8 params
# (full precision, 2B/value); the rest use single fp8 (1B/value).
_HILO16 = int(os.environ.get("KERNEL_HILO16", "10"))
# Fraction of (o,half) mul+reduce ops on DVE vs Pool: balance 679ns vs 412ns.
_DVE_OF = 5
_DVE_MOD = 13


def _is_hilo(o):
    return (o % 16) < _HILO16

_NC_CACHE = {}
LAST_EXEC_NS = None
LAST_RESULTS = None


def _np_dt(mode):
    import ml_dtypes

    return np.float16 if mode == "f16" else ml_dtypes.float8_e4m3


def _prep(x, w, bids0, bids1, matrix_perm, mode):
    """Host-side data prep: gathered-x tiles + per-core grid-ordered slabs."""
    x = np.asarray(x, np.float32)
    w = np.asarray(w, np.float32)
    bids0 = np.asarray(bids0, np.int64)
    bids1 = np.asarray(bids1, np.int64)
    mp = np.asarray(matrix_perm, np.int64)
    p0 = w[: M0 * N0].reshape(M0, N0)
    p1 = w[M0 * N0 :].reshape(M1, N1)

    colmap = np.empty(512, np.int64)
    colmap[:128] = bids0
    s_idx = np.arange(S1)
    n_idx = np.arange(N1)
    # q = 128 + s*32 + n  ->  bids1[n*12 + s]
    colmap[128:] = bids1[(n_idx[None, :] * S1 + s_idx[:, None])].reshape(384)

    dt = _np_dt(mode)
    # xg[q, b] = x[b, colmap[q]]  -> packed per chunk-pair [128, 2, 256]
    xg = np.ascontiguousarray(x[:, colmap].T, np.float32)  # (512, 256)
    xg4 = xg.reshape(KCH, 128, B)
    if mode == "f16":
        xg_t = {"xg": np.ascontiguousarray(xg4.transpose(1, 0, 2)).astype(dt)}
    else:
        xgp = np.ascontiguousarray(
            xg4.reshape(2, 2, 128, B).transpose(2, 0, 1, 3)
        )  # (128, pair, slot, B)
        hi = xgp.astype(dt)
        xg_t = {"xgh": hi}
        if mode in ("dr8c", "mix"):
            xg_t["xgl"] = (xgp - hi.astype(np.float32)).astype(dt)
        if mode == "mix":
            # per-chunk duplicated layout: [k, chunk, slot, b], same hi chunk
            # in both DoubleRow slots (pairs with pg hi/lo slots)
            xghd = np.ascontiguousarray(
                np.broadcast_to(
                    xg4.transpose(1, 0, 2)[:, :, None, :], (128, KCH, 2, B)
                )
            ).astype(dt)
            xg_t["xghd"] = xghd

    scale = PG_SCALE[mode]
    nrow = np.arange(N1)
    slabs = []
    for c in range(NCORES):
        k = mp[c * CELLS : (c + 1) * CELLS]
        pg = np.zeros((512, CELLS), np.float32)
        j0 = np.nonzero(k < M0)[0]
        pg[:128, j0] = p0[k[j0]].T * scale
        j1 = np.nonzero(k >= M0)[0]
        m1, s1 = np.divmod(k[j1] - M0, S1)
        rows = 128 + s1 * N1
        pg[(rows[:, None] + nrow[None, :]), j1[:, None]] = p1[m1] * scale
        # (512, OSH*512) -> (OSH, 128p, KCH, 512): per-o-row tile contiguous
        pg = np.ascontiguousarray(pg.reshape(KCH, 128, OSH, DIN).transpose(2, 1, 0, 3))
        if mode == "f16":
            slabs.append(pg.astype(dt).reshape(OSH, 128, KCH * DIN))
        elif mode in ("dr8", "dr8c"):
            slabs.append(pg.astype(dt).reshape(OSH, 128, 2, 2, DIN))
        else:  # mix
            hilo = np.array([_is_hilo(o) for o in range(OSH)])
            hi = pg.astype(dt)
            p8 = hi[~hilo].reshape(-1, 128, 2, 2, DIN)
            lo = (pg[hilo] - hi[hilo].astype(np.float32)).astype(dt)
            # [row, k, chunk, hi/lo, cell]
            phl = np.ascontiguousarray(
                np.stack([hi[hilo], lo], axis=3)  # (nhl, 128, KCH, 2, DIN)
            )
            slabs.append({"pg8": p8, "pghl": phl})
    return xg_t, slabs


def _prep_ef(x, w, bids0, bids1, matrix_perm):
    """e-formulation prep: compact param slabs + gathered-x/indicator units.

    Per core, cells (o,i) owned by the core are sorted type-0 first (by o),
    then type-1 by (s, o), each segment zero-padded to a shared multiple of
    128 so the device program is identical across cores. Each 128-cell tile
    gets: a compact param column block (stage A), a gathered-x tile
    xkT[cell,b] and a 0/1 indicator block Ind[cell, o_local] (stages B/C).
    """
    x = np.asarray(x, np.float32)
    w = np.asarray(w, np.float32)
    bids0 = np.asarray(bids0, np.int64)
    bids1 = np.asarray(bids1, np.int64)
    mp = np.asarray(matrix_perm, np.int64)
    p0 = w[: M0 * N0].reshape(M0, N0)
    p1 = w[M0 * N0 :].reshape(M1, N1)
    xT = np.ascontiguousarray(x.T)  # (DIN, B)

    colmap = np.empty(512, np.int64)
    colmap[:128] = bids0
    s_idx = np.arange(S1)
    n_idx = np.arange(N1)
    colmap[128:] = bids1[(n_idx[None, :] * S1 + s_idx[:, None])].reshape(384)
    xg = np.ascontiguousarray(x[:, colmap].T, np.float32)  # (512, B)
    xg0 = xg[:128].astype(np.float16)
    # type-1 gathered x: all 12 s-bands live at partitions 0-31 (nonzero PE
    # base partitions hang real hardware), indexed on the free axis
    xg1 = np.zeros((32, S1, B), np.float16)
    for s in range(S1):
        xg1[:, s, :] = xg[128 + 32 * s : 160 + 32 * s]

    # Per-core cell lists
    cores = []
    for c in range(NCORES):
        k = mp[c * CELLS : (c + 1) * CELLS]
        o = np.arange(CELLS) // DIN  # local o in [0, OSH)
        i = np.arange(CELLS) % DIN
        t0 = k < M0
        j0 = np.nonzero(t0)[0]
        # type-0 sorted by (o, i) [already in j order]
        m1, s1 = np.divmod(k[~t0] - M0, S1)
        j1 = np.nonzero(~t0)[0]
        cores.append((k, o, i, j0, (j1, m1, s1)))

    c0max = max(len(cr[3]) for cr in cores)
    C0_PAD = -(-c0max // 128) * 128
    s_counts = [
        np.bincount(cr[4][2], minlength=S1) for cr in cores
    ]
    s1max = max(int(sc.max()) for sc in s_counts)
    S_PAD = -(-s1max // 128) * 128
    ntiles = C0_PAD // 128 + S1 * (S_PAD // 128)
    if ntiles % 2:
        ntiles += 1  # final all-zero padding tile
    U = ntiles // 2

    # tilemap: tile index -> (kind, args) for the device program
    tilemap = []
    for ct in range(C0_PAD // 128):
        tilemap.append(("t0", ct))
    for s in range(S1):
        for ct in range(S_PAD // 128):
            tilemap.append(("t1", s, ct))
    while len(tilemap) < ntiles:
        tilemap.append(("pad",))

    per_core = []
    for c in range(NCORES):
        k, o, i, j0, (j1, m1, s1) = cores[c]
        pslab0 = np.zeros((128, C0_PAD), np.float16)
        pslab0[:, : len(j0)] = p0[k[j0]].T
        pslab1 = np.zeros((32, S1, S_PAD), np.float16)
        # unit slabs: [U, 128, 2, 320]: [...,0:256]=xkT f16,
        # [...,256:320] = Ind[cell, o_local] in f16
        units = np.zeros((U, 128, 2, 320), np.float16)

        def fill_tiles(base_tile, idxs):
            # idxs: global cell indices (sorted), placed at consecutive
            # positions from tile `base_tile` position 0
            n = len(idxs)
            pos = np.arange(n)
            tl = base_tile + pos // 128
            row = pos % 128
            xkvals = xT[i[idxs]]  # (n, B) f32
            units[tl // 2, row, tl % 2, :256] = xkvals.astype(np.float16)
            units[tl // 2, row, tl % 2, 256 + o[idxs]] = 1.0

        fill_tiles(0, j0)
        t1base = C0_PAD // 128
        for s in range(S1):
            sel = np.nonzero(s1 == s)[0]
            idxs = j1[sel]
            order = np.argsort(o[idxs], kind="stable")
            idxs = idxs[order]
            ms = m1[sel][order]
            pslab1[:, s, : len(idxs)] = p1[ms].T
            fill_tiles(t1base + s * (S_PAD // 128), idxs)

        per_core.append(
            {
                "pslab0": pslab0,
                "pslab1": pslab1,
                "units": units,
                "xg0": xg0,
                "xg1": xg1,
            }
        )
    return per_core, (C0_PAD, S_PAD, U, tuple(tilemap))


def _build_ef(C0_PAD, S_PAD, U, tilemap):
    import concourse.bacc as bacc
    import concourse.tile as tile
    from concourse import mybir

    f32 = mybir.dt.float32
    f16 = mybir.dt.float16

    nc = bacc.Bacc("TRN2", target_bir_lowering=False, debug=False, num_devices=NCORES)
    f8 = mybir.dt.float8e4
    ps0_d = nc.dram_tensor("pslab0", (128, C0_PAD), f16, kind="ExternalInput").ap()
    ps1_d = nc.dram_tensor("pslab1", (32, S1, S_PAD), f16, kind="ExternalInput").ap()
    xg0_d = nc.dram_tensor("xg0", (128, B), f16, kind="ExternalInput").ap()
    xg1_d = nc.dram_tensor("xg1", (32, S1, B), f16, kind="ExternalInput").ap()
    un_d = nc.dram_tensor("units", (U, 128, 2, 320), f16, kind="ExternalInput").ap()
    out_d = nc.dram_tensor("out", (OSH, B), f32, kind="ExternalOutput").ap()

    with tile.TileContext(nc) as tc:
        with (
            tc.tile_pool(name="const", bufs=1) as cp,
            tc.tile_pool(name="unp", bufs=8) as unp,
            tc.tile_pool(name="zp", bufs=6) as zp,
            tc.tile_pool(name="pse", bufs=4, space="PSUM") as pse,
            tc.tile_pool(name="pso", bufs=1, space="PSUM") as pso,
            tc.tile_pool(name="pst", bufs=1, space="PSUM") as pst,
        ):
            warmsrc = cp.tile([128, B], f32, name="warmsrc")
            nc.vector.memset(warmsrc[:], 0.0)
            warmps = pst.tile([128, 64], f32, name="warmps", tag="tp")
            for _ in range(14):
                nc.tensor.matmul(
                    warmps[:],
                    lhsT=warmsrc[:, :128],
                    rhs=warmsrc[:, :64],
                    start=True,
                    stop=True,
                )

            ps0 = cp.tile([128, C0_PAD], f16, name="ps0")
            nc.sync.dma_start(ps0[:], ps0_d)
            xg0 = cp.tile([128, B], f16, name="xg0")
            nc.sync.dma_start(xg0[:], xg0_d)
            xg1 = cp.tile([32, S1, B], f16, name="xg1")
            nc.sync.dma_start(xg1[:], xg1_d)
            # ps1 (2MB) is DMA'd lazily a few units into the loop so the
            # first type-0 units aren't stuck behind it on the DMA queue.
            ps1 = cp.tile([32, S1, S_PAD], f16, name="ps1")

            obank = pso.tile([128, B], f32, name="obank", tag="ob")
            nc.vector.memset(obank[:], 0.0)

            for u in range(U):
                un_t = unp.tile([128, 2, 320], f16, name="unt")
                nc.sync.dma_start(un_t[:], un_d[u])
                if u == 3:
                    nc.sync.dma_start(ps1[:], ps1_d)
                e_t = pse.tile([128, 2, B], f32, name="et", tag="e")
                for t in range(2):
                    kind = tilemap[2 * u + t]
                    if kind[0] == "t0":
                        ct = kind[1]
                        nc.tensor.matmul(
                            e_t[:, t, :],
                            lhsT=ps0[:, ct * 128 : (ct + 1) * 128],
                            rhs=xg0[:],
                            start=True,
                            stop=True,
                            skip_group_check=True,
                        )
                    elif kind[0] == "t1":
                        s, ct = kind[1], kind[2]
                        nc.tensor.matmul(
                            e_t[:, t, :],
                            lhsT=ps1[:, s, ct * 128 : (ct + 1) * 128],
                            rhs=xg1[:, s, :],
                            start=True,
                            stop=True,
                            skip_group_check=True,
                        )
                    else:  # pad tile: zero the psum slice via a null matmul
                        nc.tensor.matmul(
                            e_t[:, t, :],
                            lhsT=warmsrc[:, :128],
                            rhs=warmsrc[:],
                            start=True,
                            stop=True,
                            skip_group_check=True,
                        )

                # GPSIMD cannot touch PSUM on HW: Pool-path units get an
                # Activation-engine PSUM->SBUF copy first; DVE-path units
                # multiply straight out of PSUM.
                prod = zp.tile([128, 2, B], f16, name="prod")
                if (u % 9) < 4 or os.environ.get("KERNEL_EF_NOPOOL"):
                    nc.vector.scalar_tensor_tensor(
                        out=prod[:],
                        in0=e_t[:],
                        scalar=1.0,
                        in1=un_t[:, :, :256],
                        op0=mybir.AluOpType.mult,
                        op1=mybir.AluOpType.mult,
                    )
                else:
                    conv = zp.tile([128, 2, B], f16, name="conv")
                    nc.scalar.activation(
                        out=conv[:],
                        in_=e_t[:],
                        func=mybir.ActivationFunctionType.Copy,
                    )
                    nc.gpsimd.tensor_mul(
                        out=prod[:],
                        in0=conv[:],
                        in1=un_t[:, :, :256],
                    )
                for t in range(2):
                    nc.tensor.matmul(
                        obank[:64, :],
                        lhsT=un_t[:, t, 256:320],
                        rhs=prod[:, t, :],
                        start=False,
                        stop=(u == U - 1 and t == 1),
                        skip_group_check=True,
                    )

            osb = cp.tile([64, B], f32, name="osb")
            nc.scalar.activation(
                out=osb[:],
                in_=obank[:64, :],
                func=mybir.ActivationFunctionType.Copy,
                scale=0.05,
            )
            nc.sync.dma_start(out_d[:], osb[:])

    nc.compile()
    return nc


def _prep_ef2(x, w, bids0, bids1, matrix_perm):
    """ef2 prep: like ef, but cells are additionally split by o-half
    (k = o//32), the indicator is a 32-wide one-hot over o%32, and the
    param slabs are flat-concatenated per sub-segment so the device can
    fetch them in small just-in-time chunks.

    Sub-segment order: (t0,k=0), (t0,k=1), then (s, k) for s in 0..11,
    k in 0..1.  Each sub-segment is padded to a shared (across cores)
    multiple of 128 cells; every 128-cell tile maps to one matmul pair.

    Per-core tensors:
      ps0   (128, L0)      f16  type-0 param columns (tile-order)
      ps1   (32, L1)       f16  type-1 param columns (tile-order)
      xg0   (128, B)       f16  gathered x, type-0 contraction rows
      xg1   (32, S1, B)    f16  gathered x, type-1 s-bands
      units (U, 128, 2, 288) f16  per tile: 256 cols xk + 32 cols ind
    """
    x = np.asarray(x, np.float32)
    w = np.asarray(w, np.float32)
    bids0 = np.asarray(bids0, np.int64)
    bids1 = np.asarray(bids1, np.int64)
    mp = np.asarray(matrix_perm, np.int64)
    p0 = w[: M0 * N0].reshape(M0, N0)
    p1 = w[M0 * N0 :].reshape(M1, N1)
    xT16 = np.ascontiguousarray(x.T).astype(np.float16)  # (DIN, B)

    colmap = np.empty(512, np.int64)
    colmap[:128] = bids0
    s_idx = np.arange(S1)
    n_idx = np.arange(N1)
    colmap[128:] = bids1[(n_idx[None, :] * S1 + s_idx[:, None])].reshape(384)
    xg = np.ascontiguousarray(x[:, colmap].T, np.float32)  # (512, B)
    xg0 = xg[:128].astype(np.float16)
    xg1 = np.zeros((32, S1, B), np.float16)
    for s in range(S1):
        xg1[:, s, :] = xg[128 + 32 * s : 160 + 32 * s]

    # segment list: ('t0', k) x2 then ('t1', s, k)
    segs = [("t0", 0), ("t0", 1)] + [
        ("t1", s, ks) for s in range(S1) for ks in range(2)
    ]

    # per-core cell index lists per segment
    o_all = np.arange(CELLS) // DIN
    i_all = np.arange(CELLS) % DIN
    core_cells = []
    for c in range(NCORES):
        k = mp[c * CELLS : (c + 1) * CELLS]
        t0 = k < M0
        s1 = np.where(t0, -1, (k - M0) % S1)
        khalf = o_all // 32
        d = {}
        for seg in segs:
            if seg[0] == "t0":
                m = t0 & (khalf == seg[1])
            else:
                m = (s1 == seg[1]) & (khalf == seg[2])
            d[seg] = np.nonzero(m)[0]  # ascending == sorted by (o, i)
        core_cells.append(d)

    # shared tile counts per segment
    seg_tiles = {
        seg: -(-max(len(core_cells[c][seg]) for c in range(NCORES)) // 128)
        for seg in segs
    }
    ntiles = sum(seg_tiles.values())
    if ntiles % 2:
        ntiles += 1
    U = ntiles // 2

    # tilemap + slab column offsets
    tilemap = []  # per tile: ("t0"|"t1"|"pad", col_off, s_or_None, ks)
    seg_tilebase = {}
    off0 = off1 = 0
    ps1_schunk = {}  # s -> (col_off, col_len) for just-in-time DMA
    for seg in segs:
        seg_tilebase[seg] = len(tilemap)
        n128 = seg_tiles[seg] * 128
        if seg[0] == "t0":
            for ct in range(seg_tiles[seg]):
                tilemap.append(("t0", off0 + ct * 128, None, seg[1]))
            off0 += n128
        else:
            s, ks = seg[1], seg[2]
            if s not in ps1_schunk:
                ps1_schunk[s] = [off1, 0]
            ps1_schunk[s][1] += n128
            for ct in range(seg_tiles[seg]):
                tilemap.append(("t1", off1 + ct * 128, s, ks))
            off1 += n128
    while len(tilemap) < ntiles:
        tilemap.append(("pad", 0, None, 0))
    L0, L1 = max(off0, 128), max(off1, 128)

    # Just-in-time slab DMA triggers, earliest-deadline-first.  Each entry:
    # (tensor, col_off, col_len) issued on the Act queue at the given unit.
    triggers = {}
    c0 = min(1024, L0)
    extras = [("ps0", cb, min(1024, L0 - cb)) for cb in range(c0, L0, 1024)]
    extras.append(("xg1", 0, 0))
    for j, e in enumerate(extras):
        triggers.setdefault(2 * j, []).append(e)
    for s in range(S1):
        first_u = seg_tilebase[("t1", s, 0)] // 2
        tu = max(2, first_u - 10)
        triggers.setdefault(tu, []).append(("ps1",) + tuple(ps1_schunk[s]))

    per_core = []
    for c in range(NCORES):
        ps0 = np.zeros((128, L0), np.float16)
        ps1 = np.zeros((32, L1), np.float16)
        units = np.zeros((U, 128, 2, 288), np.float16)
        for seg in segs:
            idxs = core_cells[c][seg]
            n = len(idxs)
            if n == 0:
                continue
            tb = seg_tilebase[seg]
            pos = np.arange(n)
            tl = tb + pos // 128
            row = pos % 128
            units[tl // 2, row, tl % 2, :256] = xT16[i_all[idxs]]
            units[tl // 2, row, tl % 2, 256 + (o_all[idxs] % 32)] = 1.0
            if seg[0] == "t0":
                base = tilemap[tb][1]
                ps0[:, base + pos] = p0[mp[c * CELLS + idxs]].T
            else:
                base = tilemap[tb][1]
                m1 = (mp[c * CELLS + idxs] - M0) // S1
                ps1[:, base + pos] = p1[m1].T
        per_core.append(
            {"ps0": ps0, "ps1": ps1, "xg0": xg0, "xg1": xg1, "units": units}
        )
    shape_key = (L0, L1, U, tuple(tilemap), tuple(sorted(triggers.items())))
    return per_core, shape_key


def _build_ef2(L0, L1, U, tilemap, triggers_t):
    import concourse.bacc as bacc
    import concourse.tile as tile
    from concourse import mybir

    f32 = mybir.dt.float32
    f16 = mybir.dt.float16
    triggers = dict(triggers_t)

    nc = bacc.Bacc("TRN2", target_bir_lowering=False, debug=False, num_devices=NCORES)
    ps0_d = nc.dram_tensor("ps0", (128, L0), f16, kind="ExternalInput").ap()
    ps1_d = nc.dram_tensor("ps1", (32, L1), f16, kind="ExternalInput").ap()
    xg0_d = nc.dram_tensor("xg0", (128, B), f16, kind="ExternalInput").ap()
    xg1_d = nc.dram_tensor("xg1", (32, S1, B), f16, kind="ExternalInput").ap()
    un_d = nc.dram_tensor("units", (U, 128, 2, 288), f16, kind="ExternalInput").ap()
    out_d = nc.dram_tensor("out", (32, 2, B), f32, kind="ExternalOutput").ap()

    with tile.TileContext(nc) as tc:
        with (
            tc.tile_pool(name="const", bufs=1) as cp,
            tc.tile_pool(name="unp", bufs=8) as unp,
            tc.tile_pool(name="zp", bufs=9) as zp,
            tc.tile_pool(name="cvp", bufs=3) as cvp,
            tc.tile_pool(name="pse", bufs=6, space="PSUM") as pse,
            tc.tile_pool(name="pso", bufs=1, space="PSUM") as pso,
            tc.tile_pool(name="pst", bufs=1, space="PSUM") as pst,
        ):
            warmsrc = cp.tile([128, B], f16, name="warmsrc")
            nc.vector.memset(warmsrc[:], 0.0)

            # SBUF residents
            ps0_sb = cp.tile([128, L0], f16, name="ps0")
            ps1_sb = cp.tile([32, L1], f16, name="ps1")
            xg0_sb = cp.tile([128, B], f16, name="xg0")
            xg1_sb = cp.tile([32, S1, B], f16, name="xg1")

            # Preamble DMAs on the Act queue, critical-path-first (xg0 and
            # the first ps0 chunk gate unit 0); unit DMAs ride the SP queue
            # so slab transfers never head-of-line-block them.
            c0 = min(1024, L0)
            nc.sync.dma_start(xg0_sb[:], xg0_d)
            nc.sync.dma_start(ps0_sb[:, :c0], ps0_d[:, :c0])

            # One-time engine setup off the critical path: GPSIMD library
            # load + Act function-table load, triggered by dummy ops on
            # warmsrc so they don't stall the first Pool-path unit.
            dummy = cp.tile([128, 16], f16, name="dummy")
            nc.scalar.activation(
                out=dummy[:],
                in_=warmsrc[:, :16],
                func=mybir.ActivationFunctionType.Copy,
            )
            nc.gpsimd.tensor_mul(
                out=dummy[:], in0=warmsrc[:, :16], in1=warmsrc[:, :16]
            )

            # Dependency-free warm matmuls: keep the PE HAM activity window
            # busy through the DMA preamble so the clock un-throttles to
            # 8/8 before the unit loop starts (and stays there).
            warmps = pst.tile([128, 64], f32, name="warmps", tag="tp")
            for _ in range(_EF2_WARM):
                nc.tensor.matmul(
                    warmps[:],
                    lhsT=warmsrc[:, :128],
                    rhs=warmsrc[:, :64],
                    start=True,
                    stop=True,
                    skip_group_check=True,
                )

            obank = pso.tile([32, 2, B], f32, name="obank", tag="ob")
            nc.vector.memset(obank[:], 0.0)

            def emit_c(un_t, prod, kinds, last):
                for t in range(2):
                    ks = kinds[t][3]
                    nc.tensor.matmul(
                        obank[:, ks, :],
                        lhsT=un_t[:, t, 256:288],
                        rhs=prod[:, t, :],
                        start=False,
                        stop=(last and t == 1),
                        skip_group_check=True,
                    )

            pair = None
            pending = []  # pipelined stage-C work: (un_t, prod, kinds)
            for u in range(U):
                if u % 2 == 0:
                    pair = unp.tile([128, 2, 2, 288], f16, name="unpair")
                    hi = min(u + 2, U)
                    nc.sync.dma_start(
                        pair[:, : hi - u],
                        un_d[u:hi].rearrange("u p t c -> p u t c"),
                    )
                un_t = pair[:, u % 2]
                for trig in triggers.get(u, ()):
                    tid, coff, clen = trig
                    if tid == "ps1":
                        nc.sync.dma_start(
                            ps1_sb[:, coff : coff + clen],
                            ps1_d[:, coff : coff + clen],
                        )
                    elif tid == "ps0":
                        nc.sync.dma_start(
                            ps0_sb[:, coff : coff + clen],
                            ps0_d[:, coff : coff + clen],
                        )
                    else:
                        nc.sync.dma_start(xg1_sb[:], xg1_d)
                if len(pending) > _EF2_CDEPTH:
                    emit_c(*pending.pop(0), last=False)
                e_t = pse.tile([128, 2, B], f32, name="et", tag="e")
                kinds = (tilemap[2 * u], tilemap[2 * u + 1])
                for t in range(2):
                    kind = kinds[t]
                    if kind[0] == "t0":
                        nc.tensor.matmul(
                            e_t[:, t, :],
                            lhsT=ps0_sb[:, kind[1] : kind[1] + 128],
                            rhs=xg0_sb[:],
                            start=True,
                            stop=True,
                            skip_group_check=True,
                        )
                    elif kind[0] == "t1":
                        nc.tensor.matmul(
                            e_t[:, t, :],
                            lhsT=ps1_sb[:, kind[1] : kind[1] + 128],
                            rhs=xg1_sb[:, kind[2], :],
                            start=True,
                            stop=True,
                            skip_group_check=True,
                        )
                    else:  # pad tile: zero e via a null matmul
                        nc.tensor.matmul(
                            e_t[:, t, :],
                            lhsT=warmsrc[:, :128],
                            rhs=warmsrc[:],
                            start=True,
                            stop=True,
                            skip_group_check=True,
                        )

                prod = zp.tile([128, 2, B], f16, name="prod")
                if (u % _EF2_DVE_MOD) < _EF2_DVE_OF:
                    nc.vector.scalar_tensor_tensor(
                        out=prod[:],
                        in0=e_t[:],
                        scalar=1.0,
                        in1=un_t[:, :, :256],
                        op0=mybir.AluOpType.mult,
                        op1=mybir.AluOpType.mult,
                    )
                else:
                    conv = cvp.tile([128, 2, B], f16, name="conv")
                    nc.scalar.activation(
                        out=conv[:],
                        in_=e_t[:],
                        func=mybir.ActivationFunctionType.Copy,
                    )
                    nc.gpsimd.tensor_mul(
                        out=prod[:],
                        in0=conv[:],
                        in1=un_t[:, :, :256],
                    )

                pending.append((un_t, prod, kinds))
            for j, work in enumerate(pending):
                emit_c(*work, last=(j == len(pending) - 1))

            osb = cp.tile([32, 2, B], f32, name="osb")
            nc.scalar.activation(
                out=osb[:],
                in_=obank[:],
                func=mybir.ActivationFunctionType.Copy,
                scale=0.05,
            )
            nc.scalar.dma_start(out_d, osb[:])

    nc.compile()
    return nc


def _build_nc(mode):
    import concourse.bacc as bacc
    import concourse.tile as tile
    from concourse import mybir

    f32 = mybir.dt.float32
    dt_mm = mybir.dt.float16 if mode == "f16" else mybir.dt.float8e4
    dr = None if mode == "f16" else mybir.MatmulPerfMode.DoubleRow

    nc = bacc.Bacc("TRN2", target_bir_lowering=False, debug=False, num_devices=NCORES)
    x_d = nc.dram_tensor("x", (B, DIN), f32, kind="ExternalInput").ap()
    if mode == "f16":
        xg_d = {
            "xg": nc.dram_tensor("xg", (128, KCH, B), dt_mm, kind="ExternalInput").ap()
        }
        pg_d = nc.dram_tensor(
            "pg", (OSH, 128, KCH * DIN), dt_mm, kind="ExternalInput"
        ).ap()
    else:
        xg_d = {
            "xgh": nc.dram_tensor(
                "xgh", (128, 2, 2, B), dt_mm, kind="ExternalInput"
            ).ap()
        }
        if mode in ("dr8c", "mix"):
            xg_d["xgl"] = nc.dram_tensor(
                "xgl", (128, 2, 2, B), dt_mm, kind="ExternalInput"
            ).ap()
        if mode == "mix":
            xg_d["xghd"] = nc.dram_tensor(
                "xghd", (128, KCH, 2, B), dt_mm, kind="ExternalInput"
            ).ap()
            n_hl = sum(_is_hilo(o) for o in range(OSH))
            pg8_d = nc.dram_tensor(
                "pg8", (OSH - n_hl, 128, 2, 2, DIN), dt_mm, kind="ExternalInput"
            ).ap()
            pghl_d = nc.dram_tensor(
                "pghl", (n_hl, 128, KCH, 2, DIN), dt_mm, kind="ExternalInput"
            ).ap()
        else:
            pg_d = nc.dram_tensor(
                "pg", (OSH, 128, 2, 2, DIN), dt_mm, kind="ExternalInput"
            ).ap()
    out_d = nc.dram_tensor("out", (B, OSH), f32, kind="ExternalOutput").ap()

    out_scale = 0.05 / PG_SCALE[mode]

    with tile.TileContext(nc) as tc:
        with (
            tc.tile_pool(name="const", bufs=1) as cp,
            tc.tile_pool(name="pgp", bufs=6) as pgp,
            tc.tile_pool(name="zp", bufs=6) as zp,
            tc.tile_pool(name="psv", bufs=5, space="PSUM") as psv,
            tc.tile_pool(name="pst", bufs=1, space="PSUM") as pst,
        ):
            # Warm the PE clock (HAM) during the runtime preamble: dependency-free
            # matmuls on a zeroed tile, all targeting one fixed PSUM slot.
            warmsrc = cp.tile([128, 128], f32, name="warmsrc")
            nc.vector.memset(warmsrc[:], 0.0)
            warmps = pst.tile([128, 64], f32, name="warmps", tag="tp")
            for _ in range(14):
                nc.tensor.matmul(
                    warmps[:],
                    lhsT=warmsrc[:],
                    rhs=warmsrc[:, :64],
                    start=True,
                    stop=True,
                )

            x_sb2 = cp.tile([128, 2, DIN], f32, name="x2")
            nc.sync.dma_start(x_sb2[:], x_d.rearrange("(h p) i -> p h i", p=128))
            x_sb = [x_sb2[:, h, :] for h in range(2)]

            if mode == "f16":
                xg_sb = cp.tile([128, KCH, B], dt_mm, name="xg")
                nc.sync.dma_start(xg_sb[:], xg_d["xg"])
            else:
                xg_tiles = []
                names = {"dr8": ("xgh",), "dr8c": ("xgh", "xgl"), "mix": ("xgh", "xgl")}
                for name in names[mode]:
                    t = cp.tile([128, 2, 2, B], dt_mm, name=name)
                    nc.sync.dma_start(t[:], xg_d[name])
                    xg_tiles.append(t)
                if mode == "mix":
                    xghd_sb = cp.tile([128, KCH, 2, B], dt_mm, name="xghd")
                    nc.sync.dma_start(xghd_sb[:], xg_d["xghd"])
                    xgl_sb = xg_tiles[1]

            oacc = [cp.tile([128, OSH], f32, name=f"oacc{h}") for h in range(2)]

            # Row schedule: class-grouped (hilo rows then fp8 rows) so pg rows
            # can be fetched two per DMA, halving per-DMA fixed costs.
            if mode == "mix":
                sched = [(o, True) for o in range(OSH) if _is_hilo(o)]
                sched += [(o, False) for o in range(OSH) if not _is_hilo(o)]
            else:
                sched = [(o, False) for o in range(OSH)]

            pair_t = [None, None]  # current 2-row tile + slice index
            for idx, (o, hilo) in enumerate(sched):
                if mode == "f16":
                    pg_t = pgp.tile([128, KCH, DIN], dt_mm, name="pgt")
                    nc.sync.dma_start(
                        pg_t[:], pg_d[o].rearrange("p (c n) -> p c n", c=KCH)
                    )
                elif mode == "mix":
                    j = idx if hilo else idx - n_hl
                    src = pghl_d if hilo else pg8_d
                    nrows = src.shape[0]
                    if j % 2 == 0:
                        take = min(2, nrows - j)
                        shp = [128, take, KCH, 2, DIN] if hilo else [128, take, 2, 2, DIN]
                        t2 = pgp.tile(shp, dt_mm, name="pgt2")
                        nc.sync.dma_start(
                            t2[:],
                            src[j : j + take].rearrange("r p c t n -> p r c t n"),
                        )
                        pair_t = t2
                    pg_t = pair_t[:, j % 2]
                else:
                    pg_t = pgp.tile([128, 2, 2, DIN], dt_mm, name="pgt")
                    nc.sync.dma_start(pg_t[:], pg_d[o])
                for h in range(2):
                    v = psv.tile([128, DIN], f32, name="v", tag="v")
                    hs = slice(h * 128, (h + 1) * 128)
                    if mode == "f16":
                        for c in range(KCH):
                            nc.tensor.matmul(
                                v[:],
                                lhsT=xg_sb[:, c, hs],
                                rhs=pg_t[:, c, :],
                                start=(c == 0),
                                stop=(c == KCH - 1),
                            )
                    elif hilo:
                        # xgh_c*(pgh_c + pgl_c) via hi/lo in the two DR slots,
                        # then the xgl*pgh correction with chunk-paired slots.
                        for c in range(KCH):
                            nc.tensor.matmul(
                                v[:],
                                lhsT=xghd_sb[:, c, :, hs],
                                rhs=pg_t[:, c],
                                start=(c == 0),
                                stop=False,
                                perf_mode=dr,
                            )
                        for p in range(2):
                            nc.tensor.matmul(
                                v[:],
                                lhsT=xgl_sb[:, p, :, hs],
                                rhs=pg_t[:, 2 * p : 2 * p + 2, 0, :],
                                start=False,
                                stop=(p == 1),
                                perf_mode=dr,
                            )
                    else:
                        nmm = 2 * len(xg_tiles)
                        i = 0
                        for t in xg_tiles:
                            for p in range(2):
                                nc.tensor.matmul(
                                    v[:],
                                    lhsT=t[:, p, :, hs],
                                    rhs=pg_t[:, p],
                                    start=(i == 0),
                                    stop=(i == nmm - 1),
                                    perf_mode=dr,
                                )
                                i += 1
                    # fused mul+reduce: DVE reads PSUM directly; Pool path
                    # needs an Act-engine PSUM->SBUF copy first (GPSIMD
                    # cannot access PSUM on HW).
                    if (2 * o + h) % _DVE_MOD < _DVE_OF:
                        z = zp.tile([128, DIN], f32, name="z")
                        nc.vector.scalar_tensor_tensor(
                            out=z[:],
                            in0=v[:],
                            scalar=out_scale,
                            in1=x_sb[h][:],
                            op0=mybir.AluOpType.mult,
                            op1=mybir.AluOpType.mult,
                            accum_out=oacc[h][:, o : o + 1],
                        )
                    else:
                        zc = zp.tile([128, DIN], mybir.dt.bfloat16, name="zc")
                        nc.scalar.activation(
                            out=zc[:],
                            in_=v[:],
                            func=mybir.ActivationFunctionType.Copy,
                        )
                        z = zp.tile([128, DIN], f32, name="z")
                        nc.gpsimd.scalar_tensor_tensor(
                            out=z[:],
                            in0=zc[:],
                            scalar=out_scale,
                            in1=x_sb[h][:],
                            op0=mybir.AluOpType.mult,
                            op1=mybir.AluOpType.mult,
                            accum_out=oacc[h][:, o : o + 1],
                        )

            for h in range(2):
                nc.sync.dma_start(out_d[h * 128 : (h + 1) * 128, :], oacc[h][:])

    nc.compile()
    return nc


def kernel(x, w, bids0, bids1, matrix_perm):
    global LAST_EXEC_NS, LAST_RESULTS
    from concourse import bass_utils

    mode = _DT_MODE
    x = np.ascontiguousarray(np.asarray(x, np.float32))
    if mode == "ef2":
        per_core, shape_key = _prep_ef2(x, w, bids0, bids1, matrix_perm)
        key = ("ef2", shape_key[:3], _EF2_WARM, _EF2_DVE_OF, _EF2_DVE_MOD, _EF2_CDEPTH)
        if key not in _NC_CACHE:
            _NC_CACHE[key] = _build_ef2(*shape_key)
        nc = _NC_CACHE[key]
        in_maps = per_core
    elif mode == "ef":
        per_core, shape_key = _prep_ef(x, w, bids0, bids1, matrix_perm)
        key = ("ef", shape_key[:3])
        if key not in _NC_CACHE:
            _NC_CACHE[key] = _build_ef(*shape_key)
        nc = _NC_CACHE[key]
        in_maps = per_core
    else:
        xg_t, slabs = _prep(x, w, bids0, bids1, matrix_perm, mode)
        if mode not in _NC_CACHE:
            _NC_CACHE[mode] = _build_nc(mode)
        nc = _NC_CACHE[mode]
        in_maps = [
            {
                "x": x,
                **(slabs[c] if isinstance(slabs[c], dict) else {"pg": slabs[c]}),
                **xg_t,
            }
            for c in range(NCORES)
        ]
    try:
        res = bass_utils.run_bass_kernel_spmd(nc, in_maps, core_ids=list(range(NCORES)))
    except ModuleNotFoundError:
        # Tracing (BASS_TRACE=1) requires the axon NTFF hook; fall back to no-trace.
        os.environ["BASS_NEVER_TRACE"] = "1"
        res = bass_utils.run_bass_kernel_spmd(nc, in_maps, core_ids=list(range(NCORES)))
    LAST_RESULTS = res
    LAST_EXEC_NS = res.exec_time_ns

    out = np.empty((B, DOUT), np.float32)
    for c in range(NCORES):
        if mode == "ef2":
            # res["out"] is (32, 2, B): o = k*32 + r  ->  (64, B) -> (B, 64)
            o_kb = res.results[c]["out"]
            out[:, c * OSH : (c + 1) * OSH] = o_kb.transpose(1, 0, 2).reshape(
                OSH, B
            ).T
        elif mode == "ef":
            out[:, c * OSH : (c + 1) * OSH] = res.results[c]["out"].T
        else:
            out[:, c * OSH : (c + 1) * OSH] = res.results[c]["out"]
    return out



# revision 17
# speedup vs baseline: 1.0649x; 1.0649x over previous
"""Trainium2 Bass kernel for nn_BiLinear (synthetic EMLP BiLinear).

Math: out[b,o] = 0.05 * sum_i x[b,i] * Wflat[b, perm[o*512+i]]
where Wflat[b,k] is a small GEMM of param rows against gathered x columns:
  k < M0:  Wflat[b,k] = sum_{n<128} p0[k,n] * x[b, bids0[n]]
  k >= M0: (m,s) = divmod(k-M0,12); Wflat[b,k] = sum_{n<32} p1[m,n] * x[b, bids1[n*12+s]]

Since perm is a permutation of the full 512x512 (o,i) grid, we host-reorder the
param rows into "grid order": pgrid[:, o*512+i] holds the param row of cell
(o,i), scattered into a unified 512-tall contraction space
  q in [0,128)          -> gathered col bids0[q]
  q = 128 + s*32 + n    -> gathered col bids1[n*12+s]
On device (per o-row, per batch-half):
  V[b, i] = sum_q xg[q,b] * pgrid[q, (o,i)]   (accumulating matmuls)
  out[b,o] = scale * sum_i V[b,(o,i)] * x[b,i] (fused mul+reduce, balanced
                                                across DVE and Pool engines)
xg (the gathered-x operand) is host-computed and uploaded directly.

Modes:
  f16  : xg/pgrid in fp16, 4 accumulating matmuls per (o,half).
  dr8  : xg/pgrid in fp8 e4m3 (params pre-scaled x4096), chunk pairs packed
         into DoubleRow matmuls -> 2 matmuls per (o,half) at 0.5 cyc/col,
         and half the pgrid HBM traffic vs f16.
  dr8c : like dr8 but xg split into fp8 hi+lo for error compensation
         (4 DoubleRow matmuls per (o,half)).

Sharding: output rows o are split across the 8 cores (64 each). x and xg are
replicated; pgrid is partitioned. No collectives; the host concatenates the
per-core (256, 64) outputs.
"""

import os
import sys

import numpy as np

if "/opt/trn_rl_repo" not in sys.path:
    sys.path.insert(0, "/opt/trn_rl_repo")

# Problem constants (hardcoded per contract).
S0, S1 = 1, 12
N0, N1 = 128, 32
M0, M1 = 22144, 20000
DIN, DOUT = 512, 512
WSIZE = DOUT * DIN
B = 256
NCORES = 8
OSH = DOUT // NCORES  # output rows per core
KCH = 4  # contraction chunks of 128
CELLS = OSH * DIN  # grid cells per core

PG_SCALE = {"f16": 1.0, "dr8": 4096.0, "dr8c": 4096.0, "mix": 4096.0}
_DT_MODE = os.environ.get("KERNEL_DTYPE", "ef2")
# ef2 tuning knobs
_EF2_WARM = int(os.environ.get("EF2_WARM", "150"))  # warmup matmuls (N=64)
_EF2_DVE_OF = int(os.environ.get("EF2_DVE_OF", "5"))  # DVE units per MOD
_EF2_DVE_MOD = int(os.environ.get("EF2_DVE_MOD", "8"))
_EF2_CDEPTH = int(os.environ.get("EF2_CDEPTH", "7"))  # stage-C pipeline depth
# In mix mode, rows with (o % 16) < _HILO16 use hi/lo-compensated fp8 params
# (full precision, 2B/value); the rest use single fp8 (1B/value).
_HILO16 = int(os.environ.get("KERNEL_HILO16", "10"))
# Fraction of (o,half) mul+reduce ops on DVE vs Pool: balance 679ns vs 412ns.
_DVE_OF = 5
_DVE_MOD = 13


def _is_hilo(o):
    return (o % 16) < _HILO16

_NC_CACHE = {}
LAST_EXEC_NS = None
LAST_RESULTS = None


def _np_dt(mode):
    import ml_dtypes

    return np.float16 if mode == "f16" else ml_dtypes.float8_e4m3


def _prep(x, w, bids0, bids1, matrix_perm, mode):
    """Host-side data prep: gathered-x tiles + per-core grid-ordered slabs."""
    x = np.asarray(x, np.float32)
    w = np.asarray(w, np.float32)
    bids0 = np.asarray(bids0, np.int64)
    bids1 = np.asarray(bids1, np.int64)
    mp = np.asarray(matrix_perm, np.int64)
    p0 = w[: M0 * N0].reshape(M0, N0)
    p1 = w[M0 * N0 :].reshape(M1, N1)

    colmap = np.empty(512, np.int64)
    colmap[:128] = bids0
    s_idx = np.arange(S1)
    n_idx = np.arange(N1)
    # q = 128 + s*32 + n  ->  bids1[n*12 + s]
    colmap[128:] = bids1[(n_idx[None, :] * S1 + s_idx[:, None])].reshape(384)

    dt = _np_dt(mode)
    # xg[q, b] = x[b, colmap[q]]  -> packed per chunk-pair [128, 2, 256]
    xg = np.ascontiguousarray(x[:, colmap].T, np.float32)  # (512, 256)
    xg4 = xg.reshape(KCH, 128, B)
    if mode == "f16":
        xg_t = {"xg": np.ascontiguousarray(xg4.transpose(1, 0, 2)).astype(dt)}
    else:
        xgp = np.ascontiguousarray(
            xg4.reshape(2, 2, 128, B).transpose(2, 0, 1, 3)
        )  # (128, pair, slot, B)
        hi = xgp.astype(dt)
        xg_t = {"xgh": hi}
        if mode in ("dr8c", "mix"):
            xg_t["xgl"] = (xgp - hi.astype(np.float32)).astype(dt)
        if mode == "mix":
            # per-chunk duplicated layout: [k, chunk, slot, b], same hi chunk
            # in both DoubleRow slots (pairs with pg hi/lo slots)
            xghd = np.ascontiguousarray(
                np.broadcast_to(
                    xg4.transpose(1, 0, 2)[:, :, None, :], (128, KCH, 2, B)
                )
            ).astype(dt)
            xg_t["xghd"] = xghd

    scale = PG_SCALE[mode]
    nrow = np.arange(N1)
    slabs = []
    for c in range(NCORES):
        k = mp[c * CELLS : (c + 1) * CELLS]
        pg = np.zeros((512, CELLS), np.float32)
        j0 = np.nonzero(k < M0)[0]
        pg[:128, j0] = p0[k[j0]].T * scale
        j1 = np.nonzero(k >= M0)[0]
        m1, s1 = np.divmod(k[j1] - M0, S1)
        rows = 128 + s1 * N1
        pg[(rows[:, None] + nrow[None, :]), j1[:, None]] = p1[m1] * scale
        # (512, OSH*512) -> (OSH, 128p, KCH, 512): per-o-row tile contiguous
        pg = np.ascontiguousarray(pg.reshape(KCH, 128, OSH, DIN).transpose(2, 1, 0, 3))
        if mode == "f16":
            slabs.append(pg.astype(dt).reshape(OSH, 128, KCH * DIN))
        elif mode in ("dr8", "dr8c"):
            slabs.append(pg.astype(dt).reshape(OSH, 128, 2, 2, DIN))
        else:  # mix
            hilo = np.array([_is_hilo(o) for o in range(OSH)])
            hi = pg.astype(dt)
            p8 = hi[~hilo].reshape(-1, 128, 2, 2, DIN)
            lo = (pg[hilo] - hi[hilo].astype(np.float32)).astype(dt)
            # [row, k, chunk, hi/lo, cell]
            phl = np.ascontiguousarray(
                np.stack([hi[hilo], lo], axis=3)  # (nhl, 128, KCH, 2, DIN)
            )
            slabs.append({"pg8": p8, "pghl": phl})
    return xg_t, slabs


def _prep_ef(x, w, bids0, bids1, matrix_perm):
    """e-formulation prep: compact param slabs + gathered-x/indicator units.

    Per core, cells (o,i) owned by the core are sorted type-0 first (by o),
    then type-1 by (s, o), each segment zero-padded to a shared multiple of
    128 so the device program is identical across cores. Each 128-cell tile
    gets: a compact param column block (stage A), a gathered-x tile
    xkT[cell,b] and a 0/1 indicator block Ind[cell, o_local] (stages B/C).
    """
    x = np.asarray(x, np.float32)
    w = np.asarray(w, np.float32)
    bids0 = np.asarray(bids0, np.int64)
    bids1 = np.asarray(bids1, np.int64)
    mp = np.asarray(matrix_perm, np.int64)
    p0 = w[: M0 * N0].reshape(M0, N0)
    p1 = w[M0 * N0 :].reshape(M1, N1)
    xT = np.ascontiguousarray(x.T)  # (DIN, B)

    colmap = np.empty(512, np.int64)
    colmap[:128] = bids0
    s_idx = np.arange(S1)
    n_idx = np.arange(N1)
    colmap[128:] = bids1[(n_idx[None, :] * S1 + s_idx[:, None])].reshape(384)
    xg = np.ascontiguousarray(x[:, colmap].T, np.float32)  # (512, B)
    xg0 = xg[:128].astype(np.float16)
    # type-1 gathered x: all 12 s-bands live at partitions 0-31 (nonzero PE
    # base partitions hang real hardware), indexed on the free axis
    xg1 = np.zeros((32, S1, B), np.float16)
    for s in range(S1):
        xg1[:, s, :] = xg[128 + 32 * s : 160 + 32 * s]

    # Per-core cell lists
    cores = []
    for c in range(NCORES):
        k = mp[c * CELLS : (c + 1) * CELLS]
        o = np.arange(CELLS) // DIN  # local o in [0, OSH)
        i = np.arange(CELLS) % DIN
        t0 = k < M0
        j0 = np.nonzero(t0)[0]
        # type-0 sorted by (o, i) [already in j order]
        m1, s1 = np.divmod(k[~t0] - M0, S1)
        j1 = np.nonzero(~t0)[0]
        cores.append((k, o, i, j0, (j1, m1, s1)))

    c0max = max(len(cr[3]) for cr in cores)
    C0_PAD = -(-c0max // 128) * 128
    s_counts = [
        np.bincount(cr[4][2], minlength=S1) for cr in cores
    ]
    s1max = max(int(sc.max()) for sc in s_counts)
    S_PAD = -(-s1max // 128) * 128
    ntiles = C0_PAD // 128 + S1 * (S_PAD // 128)
    if ntiles % 2:
        ntiles += 1  # final all-zero padding tile
    U = ntiles // 2

    # tilemap: tile index -> (kind, args) for the device program
    tilemap = []
    for ct in range(C0_PAD // 128):
        tilemap.append(("t0", ct))
    for s in range(S1):
        for ct in range(S_PAD // 128):
            tilemap.append(("t1", s, ct))
    while len(tilemap) < ntiles:
        tilemap.append(("pad",))

    per_core = []
    for c in range(NCORES):
        k, o, i, j0, (j1, m1, s1) = cores[c]
        pslab0 = np.zeros((128, C0_PAD), np.float16)
        pslab0[:, : len(j0)] = p0[k[j0]].T
        pslab1 = np.zeros((32, S1, S_PAD), np.float16)
        # unit slabs: [U, 128, 2, 320]: [...,0:256]=xkT f16,
        # [...,256:320] = Ind[cell, o_local] in f16
        units = np.zeros((U, 128, 2, 320), np.float16)

        def fill_tiles(base_tile, idxs):
            # idxs: global cell indices (sorted), placed at consecutive
            # positions from tile `base_tile` position 0
            n = len(idxs)
            pos = np.arange(n)
            tl = base_tile + pos // 128
            row = pos % 128
            xkvals = xT[i[idxs]]  # (n, B) f32
            units[tl // 2, row, tl % 2, :256] = xkvals.astype(np.float16)
            units[tl // 2, row, tl % 2, 256 + o[idxs]] = 1.0

        fill_tiles(0, j0)
        t1base = C0_PAD // 128
        for s in range(S1):
            sel = np.nonzero(s1 == s)[0]
            idxs = j1[sel]
            order = np.argsort(o[idxs], kind="stable")
            idxs = idxs[order]
            ms = m1[sel][order]
            pslab1[:, s, : len(idxs)] = p1[ms].T
            fill_tiles(t1base + s * (S_PAD // 128), idxs)

        per_core.append(
            {
                "pslab0": pslab0,
                "pslab1": pslab1,
                "units": units,
                "xg0": xg0,
                "xg1": xg1,
            }
        )
    return per_core, (C0_PAD, S_PAD, U, tuple(tilemap))


def _build_ef(C0_PAD, S_PAD, U, tilemap):
    import concourse.bacc as bacc
    import concourse.tile as tile
    from concourse import mybir

    f32 = mybir.dt.float32
    f16 = mybir.dt.float16

    nc = bacc.Bacc("TRN2", target_bir_lowering=False, debug=False, num_devices=NCORES)
    f8 = mybir.dt.float8e4
    ps0_d = nc.dram_tensor("pslab0", (128, C0_PAD), f16, kind="ExternalInput").ap()
    ps1_d = nc.dram_tensor("pslab1", (32, S1, S_PAD), f16, kind="ExternalInput").ap()
    xg0_d = nc.dram_tensor("xg0", (128, B), f16, kind="ExternalInput").ap()
    xg1_d = nc.dram_tensor("xg1", (32, S1, B), f16, kind="ExternalInput").ap()
    un_d = nc.dram_tensor("units", (U, 128, 2, 320), f16, kind="ExternalInput").ap()
    out_d = nc.dram_tensor("out", (OSH, B), f32, kind="ExternalOutput").ap()

    with tile.TileContext(nc) as tc:
        with (
            tc.tile_pool(name="const", bufs=1) as cp,
            tc.tile_pool(name="unp", bufs=8) as unp,
            tc.tile_pool(name="zp", bufs=6) as zp,
            tc.tile_pool(name="pse", bufs=4, space="PSUM") as pse,
            tc.tile_pool(name="pso", bufs=1, space="PSUM") as pso,
        ):
            warmsrc = cp.tile([128, B], f32, name="warmsrc")
            nc.vector.memset(warmsrc[:], 0.0)
            warmps = pso.tile([128, 64], f32, name="warmps", tag="ob")
            for _ in range(14):
                nc.tensor.matmul(
                    warmps[:],
                    lhsT=warmsrc[:, :128],
                    rhs=warmsrc[:, :64],
                    start=True,
                    stop=True,
                )

            ps0 = cp.tile([128, C0_PAD], f16, name="ps0")
            nc.sync.dma_start(ps0[:], ps0_d)
            xg0 = cp.tile([128, B], f16, name="xg0")
            nc.sync.dma_start(xg0[:], xg0_d)
            xg1 = cp.tile([32, S1, B], f16, name="xg1")
            nc.sync.dma_start(xg1[:], xg1_d)
            # ps1 (2MB) is DMA'd lazily a few units into the loop so the
            # first type-0 units aren't stuck behind it on the DMA queue.
            ps1 = cp.tile([32, S1, S_PAD], f16, name="ps1")

            obank = pso.tile([128, B], f32, name="obank", tag="ob")
            nc.vector.memset(obank[:], 0.0)

            for u in range(U):
                un_t = unp.tile([128, 2, 320], f16, name="unt")
                nc.sync.dma_start(un_t[:], un_d[u])
                if u == 3:
                    nc.sync.dma_start(ps1[:], ps1_d)
                e_t = pse.tile([128, 2, B], f32, name="et", tag="e")
                for t in range(2):
                    kind = tilemap[2 * u + t]
                    if kind[0] == "t0":
                        ct = kind[1]
                        nc.tensor.matmul(
                            e_t[:, t, :],
                            lhsT=ps0[:, ct * 128 : (ct + 1) * 128],
                            rhs=xg0[:],
                            start=True,
                            stop=True,
                            skip_group_check=True,
                        )
                    elif kind[0] == "t1":
                        s, ct = kind[1], kind[2]
                        nc.tensor.matmul(
                            e_t[:, t, :],
                            lhsT=ps1[:, s, ct * 128 : (ct + 1) * 128],
                            rhs=xg1[:, s, :],
                            start=True,
                            stop=True,
                            skip_group_check=True,
                        )
                    else:  # pad tile: zero the psum slice via a null matmul
                        nc.tensor.matmul(
                            e_t[:, t, :],
                            lhsT=warmsrc[:, :128],
                            rhs=warmsrc[:],
                            start=True,
                            stop=True,
                            skip_group_check=True,
                        )

                # GPSIMD cannot touch PSUM on HW: Pool-path units get an
                # Activation-engine PSUM->SBUF copy first; DVE-path units
                # multiply straight out of PSUM.
                prod = zp.tile([128, 2, B], f16, name="prod")
                if (u % 9) < 4 or os.environ.get("KERNEL_EF_NOPOOL"):
                    nc.vector.scalar_tensor_tensor(
                        out=prod[:],
                        in0=e_t[:],
                        scalar=1.0,
                        in1=un_t[:, :, :256],
                        op0=mybir.AluOpType.mult,
                        op1=mybir.AluOpType.mult,
                    )
                else:
                    conv = zp.tile([128, 2, B], f16, name="conv")
                    nc.scalar.activation(
                        out=conv[:],
                        in_=e_t[:],
                        func=mybir.ActivationFunctionType.Copy,
                    )
                    nc.gpsimd.tensor_mul(
                        out=prod[:],
                        in0=conv[:],
                        in1=un_t[:, :, :256],
                    )
                for t in range(2):
                    nc.tensor.matmul(
                        obank[:64, :],
                        lhsT=un_t[:, t, 256:320],
                        rhs=prod[:, t, :],
                        start=False,
                        stop=(u == U - 1 and t == 1),
                        skip_group_check=True,
                    )

            osb = cp.tile([64, B], f32, name="osb")
            nc.scalar.activation(
                out=osb[:],
                in_=obank[:64, :],
                func=mybir.ActivationFunctionType.Copy,
                scale=0.05,
            )
            nc.sync.dma_start(out_d[:], osb[:])

    nc.compile()
    return nc


def _prep_ef2(x, w, bids0, bids1, matrix_perm):
    """ef2 prep: like ef, but cells are additionally split by o-half
    (k = o//32), the indicator is a 32-wide one-hot over o%32, and the
    param slabs are flat-concatenated per sub-segment so the device can
    fetch them in small just-in-time chunks.

    Sub-segment order: (t0,k=0), (t0,k=1), then (s, k) for s in 0..11,
    k in 0..1.  Each sub-segment is padded to a shared (across cores)
    multiple of 128 cells; every 128-cell tile maps to one matmul pair.

    Per-core tensors:
      ps0   (128, L0)      f16  type-0 param columns (tile-order)
      ps1   (32, L1)       f16  type-1 param columns (tile-order)
      xg0   (128, B)       f16  gathered x, type-0 contraction rows
      xg1   (32, S1, B)    f16  gathered x, type-1 s-bands
      units (U, 128, 2, 288) f16  per tile: 256 cols xk + 32 cols ind
    """
    x = np.asarray(x, np.float32)
    w = np.asarray(w, np.float32)
    bids0 = np.asarray(bids0, np.int64)
    bids1 = np.asarray(bids1, np.int64)
    mp = np.asarray(matrix_perm, np.int64)
    p0 = w[: M0 * N0].reshape(M0, N0)
    p1 = w[M0 * N0 :].reshape(M1, N1)
    xT16 = np.ascontiguousarray(x.T).astype(np.float16)  # (DIN, B)

    colmap = np.empty(512, np.int64)
    colmap[:128] = bids0
    s_idx = np.arange(S1)
    n_idx = np.arange(N1)
    colmap[128:] = bids1[(n_idx[None, :] * S1 + s_idx[:, None])].reshape(384)
    xg = np.ascontiguousarray(x[:, colmap].T, np.float32)  # (512, B)
    xg0 = xg[:128].astype(np.float16)
    xg1 = np.zeros((32, S1, B), np.float16)
    for s in range(S1):
        xg1[:, s, :] = xg[128 + 32 * s : 160 + 32 * s]

    # segment list: ('t0', k) x2 then ('t1', s, k)
    segs = [("t0", 0), ("t0", 1)] + [
        ("t1", s, ks) for s in range(S1) for ks in range(2)
    ]

    # per-core cell index lists per segment
    o_all = np.arange(CELLS) // DIN
    i_all = np.arange(CELLS) % DIN
    core_cells = []
    for c in range(NCORES):
        k = mp[c * CELLS : (c + 1) * CELLS]
        t0 = k < M0
        s1 = np.where(t0, -1, (k - M0) % S1)
        khalf = o_all // 32
        d = {}
        for seg in segs:
            if seg[0] == "t0":
                m = t0 & (khalf == seg[1])
            else:
                m = (s1 == seg[1]) & (khalf == seg[2])
            d[seg] = np.nonzero(m)[0]  # ascending == sorted by (o, i)
        core_cells.append(d)

    # shared tile counts per segment
    seg_tiles = {
        seg: -(-max(len(core_cells[c][seg]) for c in range(NCORES)) // 128)
        for seg in segs
    }
    ntiles = sum(seg_tiles.values())
    if ntiles % 2:
        ntiles += 1
    U = ntiles // 2

    # tilemap + slab column offsets
    tilemap = []  # per tile: ("t0"|"t1"|"pad", col_off, s_or_None, ks)
    seg_tilebase = {}
    off0 = off1 = 0
    ps1_schunk = {}  # s -> (col_off, col_len) for just-in-time DMA
    for seg in segs:
        seg_tilebase[seg] = len(tilemap)
        n128 = seg_tiles[seg] * 128
        if seg[0] == "t0":
            for ct in range(seg_tiles[seg]):
                tilemap.append(("t0", off0 + ct * 128, None, seg[1]))
            off0 += n128
        else:
            s, ks = seg[1], seg[2]
            if s not in ps1_schunk:
                ps1_schunk[s] = [off1, 0]
            ps1_schunk[s][1] += n128
            for ct in range(seg_tiles[seg]):
                tilemap.append(("t1", off1 + ct * 128, s, ks))
            off1 += n128
    while len(tilemap) < ntiles:
        tilemap.append(("pad", 0, None, 0))
    L0, L1 = max(off0, 128), max(off1, 128)

    # Just-in-time slab DMA triggers, earliest-deadline-first.  Each entry:
    # (tensor, col_off, col_len) issued on the Act queue at the given unit.
    triggers = {}
    c0 = min(1024, L0)
    extras = [("ps0", cb, min(1024, L0 - cb)) for cb in range(c0, L0, 1024)]
    extras.append(("xg1", 0, 0))
    for j, e in enumerate(extras):
        triggers.setdefault(2 * j, []).append(e)
    for s in range(S1):
        first_u = seg_tilebase[("t1", s, 0)] // 2
        tu = max(2, first_u - 10)
        triggers.setdefault(tu, []).append(("ps1",) + tuple(ps1_schunk[s]))

    per_core = []
    for c in range(NCORES):
        ps0 = np.zeros((128, L0), np.float16)
        ps1 = np.zeros((32, L1), np.float16)
        units = np.zeros((U, 128, 2, 288), np.float16)
        for seg in segs:
            idxs = core_cells[c][seg]
            n = len(idxs)
            if n == 0:
                continue
            tb = seg_tilebase[seg]
            pos = np.arange(n)
            tl = tb + pos // 128
            row = pos % 128
            units[tl // 2, row, tl % 2, :256] = xT16[i_all[idxs]]
            units[tl // 2, row, tl % 2, 256 + (o_all[idxs] % 32)] = 1.0
            if seg[0] == "t0":
                base = tilemap[tb][1]
                ps0[:, base + pos] = p0[mp[c * CELLS + idxs]].T
            else:
                base = tilemap[tb][1]
                m1 = (mp[c * CELLS + idxs] - M0) // S1
                ps1[:, base + pos] = p1[m1].T
        per_core.append(
            {"ps0": ps0, "ps1": ps1, "xg0": xg0, "xg1": xg1, "units": units}
        )
    shape_key = (L0, L1, U, tuple(tilemap), tuple(sorted(triggers.items())))
    return per_core, shape_key


def _build_ef2(L0, L1, U, tilemap, triggers_t):
    import concourse.bacc as bacc
    import concourse.tile as tile
    from concourse import mybir

    f32 = mybir.dt.float32
    f16 = mybir.dt.float16
    triggers = dict(triggers_t)

    nc = bacc.Bacc("TRN2", target_bir_lowering=False, debug=False, num_devices=NCORES)
    ps0_d = nc.dram_tensor("ps0", (128, L0), f16, kind="ExternalInput").ap()
    ps1_d = nc.dram_tensor("ps1", (32, L1), f16, kind="ExternalInput").ap()
    xg0_d = nc.dram_tensor("xg0", (128, B), f16, kind="ExternalInput").ap()
    xg1_d = nc.dram_tensor("xg1", (32, S1, B), f16, kind="ExternalInput").ap()
    un_d = nc.dram_tensor("units", (U, 128, 2, 288), f16, kind="ExternalInput").ap()
    out_d = nc.dram_tensor("out", (32, 2, B), f32, kind="ExternalOutput").ap()

    with tile.TileContext(nc) as tc:
        with (
            tc.tile_pool(name="const", bufs=1) as cp,
            tc.tile_pool(name="unp", bufs=8) as unp,
            tc.tile_pool(name="zp", bufs=10) as zp,
            tc.tile_pool(name="cvp", bufs=3) as cvp,
            tc.tile_pool(name="pse", bufs=7, space="PSUM") as pse,
            tc.tile_pool(name="pso", bufs=1, space="PSUM") as pso,
        ):
            warmsrc = cp.tile([128, B], f16, name="warmsrc")
            nc.vector.memset(warmsrc[:], 0.0)

            # SBUF residents
            ps0_sb = cp.tile([128, L0], f16, name="ps0")
            ps1_sb = cp.tile([32, L1], f16, name="ps1")
            xg0_sb = cp.tile([128, B], f16, name="xg0")
            xg1_sb = cp.tile([32, S1, B], f16, name="xg1")

            # Preamble DMAs on the Act queue, critical-path-first (xg0 and
            # the first ps0 chunk gate unit 0); unit DMAs ride the SP queue
            # so slab transfers never head-of-line-block them.
            c0 = min(1024, L0)
            nc.sync.dma_start(xg0_sb[:], xg0_d)
            nc.sync.dma_start(ps0_sb[:, :c0], ps0_d[:, :c0])

            # One-time engine setup off the critical path: GPSIMD library
            # load + Act function-table load, triggered by dummy ops on
            # warmsrc so they don't stall the first Pool-path unit.
            dummy = cp.tile([128, 16], f16, name="dummy")
            nc.scalar.activation(
                out=dummy[:],
                in_=warmsrc[:, :16],
                func=mybir.ActivationFunctionType.Copy,
            )
            nc.gpsimd.tensor_mul(
                out=dummy[:], in0=warmsrc[:, :16], in1=warmsrc[:, :16]
            )

            # Dependency-free warm matmuls: keep the PE HAM activity window
            # busy through the DMA preamble so the clock un-throttles to
            # 8/8 before the unit loop starts (and stays there).
            warmps = pso.tile([128, 64], f32, name="warmps", tag="ob")
            for _ in range(_EF2_WARM):
                nc.tensor.matmul(
                    warmps[:],
                    lhsT=warmsrc[:, :128],
                    rhs=warmsrc[:, :64],
                    start=True,
                    stop=True,
                    skip_group_check=True,
                )

            obank = pso.tile([32, 2, B], f32, name="obank", tag="ob")
            nc.vector.memset(obank[:], 0.0)

            def emit_c(un_t, prod, kinds, last):
                for t in range(2):
                    ks = kinds[t][3]
                    nc.tensor.matmul(
                        obank[:, ks, :],
                        lhsT=un_t[:, t, 256:288],
                        rhs=prod[:, t, :],
                        start=False,
                        stop=(last and t == 1),
                        skip_group_check=True,
                    )

            pair = None
            pending = []  # pipelined stage-C work: (un_t, prod, kinds)
            for u in range(U):
                if u % 2 == 0:
                    pair = unp.tile([128, 2, 2, 288], f16, name="unpair")
                    hi = min(u + 2, U)
                    nc.sync.dma_start(
                        pair[:, : hi - u],
                        un_d[u:hi].rearrange("u p t c -> p u t c"),
                    )
                un_t = pair[:, u % 2]
                for trig in triggers.get(u, ()):
                    tid, coff, clen = trig
                    if tid == "ps1":
                        nc.sync.dma_start(
                            ps1_sb[:, coff : coff + clen],
                            ps1_d[:, coff : coff + clen],
                        )
                    elif tid == "ps0":
                        nc.sync.dma_start(
                            ps0_sb[:, coff : coff + clen],
                            ps0_d[:, coff : coff + clen],
                        )
                    else:
                        nc.sync.dma_start(xg1_sb[:], xg1_d)
                if len(pending) > _EF2_CDEPTH:
                    emit_c(*pending.pop(0), last=False)
                e_t = pse.tile([128, 2, B], f32, name="et", tag="e")
                kinds = (tilemap[2 * u], tilemap[2 * u + 1])
                for t in range(2):
                    kind = kinds[t]
                    if kind[0] == "t0":
                        nc.tensor.matmul(
                            e_t[:, t, :],
                            lhsT=ps0_sb[:, kind[1] : kind[1] + 128],
                            rhs=xg0_sb[:],
                            start=True,
                            stop=True,
                            skip_group_check=True,
                        )
                    elif kind[0] == "t1":
                        nc.tensor.matmul(
                            e_t[:, t, :],
                            lhsT=ps1_sb[:, kind[1] : kind[1] + 128],
                            rhs=xg1_sb[:, kind[2], :],
                            start=True,
                            stop=True,
                            skip_group_check=True,
                        )
                    else:  # pad tile: zero e via a null matmul
                        nc.tensor.matmul(
                            e_t[:, t, :],
                            lhsT=warmsrc[:, :128],
                            rhs=warmsrc[:],
                            start=True,
                            stop=True,
                            skip_group_check=True,
                        )

                prod = zp.tile([128, 2, B], f16, name="prod")
                if (u % _EF2_DVE_MOD) not in (0, 3, 6):
                    nc.vector.scalar_tensor_tensor(
                        out=prod[:],
                        in0=e_t[:],
                        scalar=1.0,
                        in1=un_t[:, :, :256],
                        op0=mybir.AluOpType.mult,
                        op1=mybir.AluOpType.mult,
                    )
                else:
                    conv = cvp.tile([128, 2, B], f16, name="conv")
                    nc.scalar.activation(
                        out=conv[:],
                        in_=e_t[:],
                        func=mybir.ActivationFunctionType.Copy,
                    )
                    nc.gpsimd.tensor_mul(
                        out=prod[:],
                        in0=conv[:],
                        in1=un_t[:, :, :256],
                    )

                pending.append((un_t, prod, kinds))
            for j, work in enumerate(pending):
                emit_c(*work, last=(j == len(pending) - 1))

            osb = cp.tile([32, 2, B], f32, name="osb")
            nc.scalar.activation(
                out=osb[:],
                in_=obank[:],
                func=mybir.ActivationFunctionType.Copy,
                scale=0.05,
            )
            nc.scalar.dma_start(out_d, osb[:])

    nc.compile()
    return nc


def _build_nc(mode):
    import concourse.bacc as bacc
    import concourse.tile as tile
    from concourse import mybir

    f32 = mybir.dt.float32
    dt_mm = mybir.dt.float16 if mode == "f16" else mybir.dt.float8e4
    dr = None if mode == "f16" else mybir.MatmulPerfMode.DoubleRow

    nc = bacc.Bacc("TRN2", target_bir_lowering=False, debug=False, num_devices=NCORES)
    x_d = nc.dram_tensor("x", (B, DIN), f32, kind="ExternalInput").ap()
    if mode == "f16":
        xg_d = {
            "xg": nc.dram_tensor("xg", (128, KCH, B), dt_mm, kind="ExternalInput").ap()
        }
        pg_d = nc.dram_tensor(
            "pg", (OSH, 128, KCH * DIN), dt_mm, kind="ExternalInput"
        ).ap()
    else:
        xg_d = {
            "xgh": nc.dram_tensor(
                "xgh", (128, 2, 2, B), dt_mm, kind="ExternalInput"
            ).ap()
        }
        if mode in ("dr8c", "mix"):
            xg_d["xgl"] = nc.dram_tensor(
                "xgl", (128, 2, 2, B), dt_mm, kind="ExternalInput"
            ).ap()
        if mode == "mix":
            xg_d["xghd"] = nc.dram_tensor(
                "xghd", (128, KCH, 2, B), dt_mm, kind="ExternalInput"
            ).ap()
            n_hl = sum(_is_hilo(o) for o in range(OSH))
            pg8_d = nc.dram_tensor(
                "pg8", (OSH - n_hl, 128, 2, 2, DIN), dt_mm, kind="ExternalInput"
            ).ap()
            pghl_d = nc.dram_tensor(
                "pghl", (n_hl, 128, KCH, 2, DIN), dt_mm, kind="ExternalInput"
            ).ap()
        else:
            pg_d = nc.dram_tensor(
                "pg", (OSH, 128, 2, 2, DIN), dt_mm, kind="ExternalInput"
            ).ap()
    out_d = nc.dram_tensor("out", (B, OSH), f32, kind="ExternalOutput").ap()

    out_scale = 0.05 / PG_SCALE[mode]

    with tile.TileContext(nc) as tc:
        with (
            tc.tile_pool(name="const", bufs=1) as cp,
            tc.tile_pool(name="pgp", bufs=6) as pgp,
            tc.tile_pool(name="zp", bufs=6) as zp,
            tc.tile_pool(name="psv", bufs=5, space="PSUM") as psv,
            tc.tile_pool(name="pst", bufs=1, space="PSUM") as pst,
        ):
            # Warm the PE clock (HAM) during the runtime preamble: dependency-free
            # matmuls on a zeroed tile, all targeting one fixed PSUM slot.
            warmsrc = cp.tile([128, 128], f32, name="warmsrc")
            nc.vector.memset(warmsrc[:], 0.0)
            warmps = pso.tile([128, 64], f32, name="warmps", tag="ob")
            for _ in range(14):
                nc.tensor.matmul(
                    warmps[:],
                    lhsT=warmsrc[:],
                    rhs=warmsrc[:, :64],
                    start=True,
                    stop=True,
                )

            x_sb2 = cp.tile([128, 2, DIN], f32, name="x2")
            nc.sync.dma_start(x_sb2[:], x_d.rearrange("(h p) i -> p h i", p=128))
            x_sb = [x_sb2[:, h, :] for h in range(2)]

            if mode == "f16":
                xg_sb = cp.tile([128, KCH, B], dt_mm, name="xg")
                nc.sync.dma_start(xg_sb[:], xg_d["xg"])
            else:
                xg_tiles = []
                names = {"dr8": ("xgh",), "dr8c": ("xgh", "xgl"), "mix": ("xgh", "xgl")}
                for name in names[mode]:
                    t = cp.tile([128, 2, 2, B], dt_mm, name=name)
                    nc.sync.dma_start(t[:], xg_d[name])
                    xg_tiles.append(t)
                if mode == "mix":
                    xghd_sb = cp.tile([128, KCH, 2, B], dt_mm, name="xghd")
                    nc.sync.dma_start(xghd_sb[:], xg_d["xghd"])
                    xgl_sb = xg_tiles[1]

            oacc = [cp.tile([128, OSH], f32, name=f"oacc{h}") for h in range(2)]

            # Row schedule: class-grouped (hilo rows then fp8 rows) so pg rows
            # can be fetched two per DMA, halving per-DMA fixed costs.
            if mode == "mix":
                sched = [(o, True) for o in range(OSH) if _is_hilo(o)]
                sched += [(o, False) for o in range(OSH) if not _is_hilo(o)]
            else:
                sched = [(o, False) for o in range(OSH)]

            pair_t = [None, None]  # current 2-row tile + slice index
            for idx, (o, hilo) in enumerate(sched):
                if mode == "f16":
                    pg_t = pgp.tile([128, KCH, DIN], dt_mm, name="pgt")
                    nc.sync.dma_start(
                        pg_t[:], pg_d[o].rearrange("p (c n) -> p c n", c=KCH)
                    )
                elif mode == "mix":
                    j = idx if hilo else idx - n_hl
                    src = pghl_d if hilo else pg8_d
                    nrows = src.shape[0]
                    if j % 2 == 0:
                        take = min(2, nrows - j)
                        shp = [128, take, KCH, 2, DIN] if hilo else [128, take, 2, 2, DIN]
                        t2 = pgp.tile(shp, dt_mm, name="pgt2")
                        nc.sync.dma_start(
                            t2[:],
                            src[j : j + take].rearrange("r p c t n -> p r c t n"),
                        )
                        pair_t = t2
                    pg_t = pair_t[:, j % 2]
                else:
                    pg_t = pgp.tile([128, 2, 2, DIN], dt_mm, name="pgt")
                    nc.sync.dma_start(pg_t[:], pg_d[o])
                for h in range(2):
                    v = psv.tile([128, DIN], f32, name="v", tag="v")
                    hs = slice(h * 128, (h + 1) * 128)
                    if mode == "f16":
                        for c in range(KCH):
                            nc.tensor.matmul(
                                v[:],
                                lhsT=xg_sb[:, c, hs],
                                rhs=pg_t[:, c, :],
                                start=(c == 0),
                                stop=(c == KCH - 1),
                            )
                    elif hilo:
                        # xgh_c*(pgh_c + pgl_c) via hi/lo in the two DR slots,
                        # then the xgl*pgh correction with chunk-paired slots.
                        for c in range(KCH):
                            nc.tensor.matmul(
                                v[:],
                                lhsT=xghd_sb[:, c, :, hs],
                                rhs=pg_t[:, c],
                                start=(c == 0),
                                stop=False,
                                perf_mode=dr,
                            )
                        for p in range(2):
                            nc.tensor.matmul(
                                v[:],
                                lhsT=xgl_sb[:, p, :, hs],
                                rhs=pg_t[:, 2 * p : 2 * p + 2, 0, :],
                                start=False,
                                stop=(p == 1),
                                perf_mode=dr,
                            )
                    else:
                        nmm = 2 * len(xg_tiles)
                        i = 0
                        for t in xg_tiles:
                            for p in range(2):
                                nc.tensor.matmul(
                                    v[:],
                                    lhsT=t[:, p, :, hs],
                                    rhs=pg_t[:, p],
                                    start=(i == 0),
                                    stop=(i == nmm - 1),
                                    perf_mode=dr,
                                )
                                i += 1
                    # fused mul+reduce: DVE reads PSUM directly; Pool path
                    # needs an Act-engine PSUM->SBUF copy first (GPSIMD
                    # cannot access PSUM on HW).
                    if (2 * o + h) % _DVE_MOD < _DVE_OF:
                        z = zp.tile([128, DIN], f32, name="z")
                        nc.vector.scalar_tensor_tensor(
                            out=z[:],
                            in0=v[:],
                            scalar=out_scale,
                            in1=x_sb[h][:],
                            op0=mybir.AluOpType.mult,
                            op1=mybir.AluOpType.mult,
                            accum_out=oacc[h][:, o : o + 1],
                        )
                    else:
                        zc = zp.tile([128, DIN], mybir.dt.bfloat16, name="zc")
                        nc.scalar.activation(
                            out=zc[:],
                            in_=v[:],
                            func=mybir.ActivationFunctionType.Copy,
                        )
                        z = zp.tile([128, DIN], f32, name="z")
                        nc.gpsimd.scalar_tensor_tensor(
                            out=z[:],
                            in0=zc[:],
                            scalar=out_scale,
                            in1=x_sb[h][:],
                            op0=mybir.AluOpType.mult,
                            op1=mybir.AluOpType.mult,
                            accum_out=oacc[h][:, o : o + 1],
                        )

            for h in range(2):
                nc.sync.dma_start(out_d[h * 128 : (h + 1) * 128, :], oacc[h][:])

    nc.compile()
    return nc


def kernel(x, w, bids0, bids1, matrix_perm):
    global LAST_EXEC_NS, LAST_RESULTS
    from concourse import bass_utils

    mode = _DT_MODE
    x = np.ascontiguousarray(np.asarray(x, np.float32))
    if mode == "ef2":
        per_core, shape_key = _prep_ef2(x, w, bids0, bids1, matrix_perm)
        key = ("ef2", shape_key[:3], _EF2_WARM, _EF2_DVE_OF, _EF2_DVE_MOD, _EF2_CDEPTH)
        if key not in _NC_CACHE:
            _NC_CACHE[key] = _build_ef2(*shape_key)
        nc = _NC_CACHE[key]
        in_maps = per_core
    elif mode == "ef":
        per_core, shape_key = _prep_ef(x, w, bids0, bids1, matrix_perm)
        key = ("ef", shape_key[:3])
        if key not in _NC_CACHE:
            _NC_CACHE[key] = _build_ef(*shape_key)
        nc = _NC_CACHE[key]
        in_maps = per_core
    else:
        xg_t, slabs = _prep(x, w, bids0, bids1, matrix_perm, mode)
        if mode not in _NC_CACHE:
            _NC_CACHE[mode] = _build_nc(mode)
        nc = _NC_CACHE[mode]
        in_maps = [
            {
                "x": x,
                **(slabs[c] if isinstance(slabs[c], dict) else {"pg": slabs[c]}),
                **xg_t,
            }
            for c in range(NCORES)
        ]
    try:
        res = bass_utils.run_bass_kernel_spmd(nc, in_maps, core_ids=list(range(NCORES)))
    except ModuleNotFoundError:
        # Tracing (BASS_TRACE=1) requires the axon NTFF hook; fall back to no-trace.
        os.environ["BASS_NEVER_TRACE"] = "1"
        res = bass_utils.run_bass_kernel_spmd(nc, in_maps, core_ids=list(range(NCORES)))
    LAST_RESULTS = res
    LAST_EXEC_NS = res.exec_time_ns

    out = np.empty((B, DOUT), np.float32)
    for c in range(NCORES):
        if mode == "ef2":
            # res["out"] is (32, 2, B): o = k*32 + r  ->  (64, B) -> (B, 64)
            o_kb = res.results[c]["out"]
            out[:, c * OSH : (c + 1) * OSH] = o_kb.transpose(1, 0, 2).reshape(
                OSH, B
            ).T
        elif mode == "ef":
            out[:, c * OSH : (c + 1) * OSH] = res.results[c]["out"].T
        else:
            out[:, c * OSH : (c + 1) * OSH] = res.results[c]["out"]
    return out

